# revision 32
# baseline (speedup 1.0000x reference)
"""AverageDistanceLoss (ADD / ADD-S with margin) on 8 Trainium2 NeuronCores.

Math (reference semantics):
  per ROI b with label l>0, R1=quat_to_rotmat(pred), R2=quat_to_rotmat(target),
  pts = points[l] (a_q columns), x1 = R1 a, x2 = R2 a:
    non-sym: d[p] = ||(R1 - R2) a_p||^2
    sym:     d[p] = min_q (||R1 a_p||^2 + n2[q] - 2 a_p^T (R1^T R2) a_q)
             with n2[q] = ||R2 a_q||^2 = s2 . u_q  (u = quadratic features)
  loss = sum_b,p max(0.5 d - 0.01, 0) / (B*P)

Device strategy (per core, SPMD over 8 cores):
  - host shards valid ROIs: symmetric and non-symmetric ROIs round-robin across
    cores, padded with identity-quaternion slots (contribute exactly 0).
  - host sends per-slot tables: a = pts^T [3,P], u = quadratic features [6,P],
    tp = packed [a;1] strips for two PE row-groups, and quats q.
  - device computes R1, R2, G=-2 R1^T R2, S2=R2^T R2 batched on partitions,
    transposes them into per-slot lhsT master tiles. Per symmetric slot:
      Y4[0:3] = G a, Y4[3] = s2 . u   (two accumulating f32r matmuls)
      M[p,q]  = [a;1]^T Y4            (pairwise, 2-way row-group packed)
      minq    = reduce_min over q     (DVE, from PSUM)
      n1[p]   = ||R1 a_p||^2          (packed tiny matmuls + Square + reduce)
      hinge   = Relu(0.5*(n1+minq) - 0.01)
    per non-symmetric slot: hinge = Relu(0.5*||(R1-R2)a_p||^2 - 0.01).
  - per-core scalar = partition-sum of all hinges; host sums 8 scalars / B*P.
"""
import sys
import types
import numpy as np
from contextlib import ExitStack

import concourse.tile as tile
from concourse import bacc, mybir
from concourse.bass_utils import run_bass_kernel_spmd

F32 = mybir.dt.float32
F32R = mybir.dt.float32r
BF16 = mybir.dt.bfloat16
AX = mybir.AxisListType
ALU = mybir.AluOpType
ACTF = mybir.ActivationFunctionType

N_CORES = 8
B, C, P = 128, 22, 1024
MARGIN = 0.01

# ---------------------------------------------------------------------------
# Optional NTFF profiling support (used by test.py via BASS_TRACE=1).
# The agent image lacks antenv.axon_hooks; provide it so trace=True works.
try:
    import antenv.axon_hooks  # noqa: F401
except ImportError:
    _hooks = types.ModuleType("antenv.axon_hooks")
    _hook_store = [None]
    _hooks.set_axon_ntff_profile_hook = lambda h: _hook_store.__setitem__(0, h)
    _hooks.get_axon_ntff_profile_hook = lambda: _hook_store[0]
    sys.modules["antenv.axon_hooks"] = _hooks

    def _try_install_ntff_hook():
        try:
            from trn_agent_boot.trn_boot import _ntff_profile_via_ctypes
            h = _ntff_profile_via_ctypes("/opt/axon/libaxon_pjrt.so")
            if h is not None:
                _hooks.set_axon_ntff_profile_hook(h)
        except Exception:
            pass

    _try_install_ntff_hook()

last_results = None  # BassKernelResults of the most recent run (for test.py)

_program_cache = {}


def _emit_quat_entries(nc, Re, Pt):
    """Re[:, 3j+i] = R[i, j] from pre-2x-scaled products Pt
    (cols: uu,vv,ww,uv,uw,vw,su,sv,sw, all already scaled by 2)."""
    v = nc.vector
    # diag pre-sums: vv+ww, uu+ww, uu+vv
    D = Re[:, 12:15]  # scratch columns (junk area of the 32-wide tile)
    v.tensor_add(D[:, 0:1], Pt[:, 1:2], Pt[:, 2:3])
    v.tensor_add(D[:, 1:2], Pt[:, 0:1], Pt[:, 2:3])
    v.tensor_add(D[:, 2:3], Pt[:, 0:1], Pt[:, 1:2])
    for e, c in ((0, 0), (4, 1), (8, 2)):  # 1 - (sum)
        nc.scalar.activation(Re[:, e:e + 1], D[:, c:c + 1], ACTF.Copy,
                             bias=1.0, scale=-1.0)
    # off-diag: e = 3j+i holds R[i,j]
    v.tensor_add(Re[:, 1:2], Pt[:, 3:4], Pt[:, 8:9])   # R[1,0] = uv+sw
    v.tensor_sub(Re[:, 3:4], Pt[:, 3:4], Pt[:, 8:9])   # R[0,1] = uv-sw
    v.tensor_sub(Re[:, 2:3], Pt[:, 4:5], Pt[:, 7:8])   # R[2,0] = uw-sv
    v.tensor_add(Re[:, 6:7], Pt[:, 4:5], Pt[:, 7:8])   # R[0,2] = uw+sv
    v.tensor_add(Re[:, 5:6], Pt[:, 5:6], Pt[:, 6:7])   # R[2,1] = vw+su
    v.tensor_sub(Re[:, 7:8], Pt[:, 5:6], Pt[:, 6:7])   # R[1,2] = vw-su


def build_program(S, NS):
    """Build the SPMD program for S symmetric + NS non-symmetric slots/core."""
    R = S + NS
    assert 1 <= R <= 32
    nc = bacc.Bacc("TRN2", target_bir_lowering=False, debug=False,
                   num_devices=N_CORES)
    q_in = nc.declare_dram_parameter("q", [32, 8], F32, isOutput=False)
    tau_in = nc.declare_dram_parameter("tau", [R, 9, P], F32R, isOutput=False)
    tp_in = nc.declare_dram_parameter("tp", [R, 4, P], BF16, isOutput=False)
    up_in = nc.declare_dram_parameter("up", [R, 128, 48], F32, isOutput=False)
    out_d = nc.declare_dram_parameter("out", [1], F32, isOutput=True)

    with tile.TileContext(nc) as tc:
        with ExitStack() as ctx:
            sing = ctx.enter_context(tc.tile_pool(name="sing", bufs=1))
            mats = ctx.enter_context(tc.tile_pool(name="mats", bufs=1))
            work = ctx.enter_context(tc.tile_pool(name="work", bufs=4))
            ybp = ctx.enter_context(tc.tile_pool(name="ybp", bufs=4))
            pwp = ctx.enter_context(tc.tile_pool(name="pwp", bufs=3,
                                                 space="PSUM"))
            ypp = ctx.enter_context(tc.tile_pool(name="ypp", bufs=1,
                                                 space="PSUM"))
            wpp = ctx.enter_context(tc.tile_pool(name="wpp", bufs=1,
                                                 space="PSUM"))

            # ---- quaternion input first: it heads the gpsimd queue and
            # gates the whole masters chain --------------------------------
            Q = sing.tile([32, 8], F32, tag="Q")
            nc.gpsimd.dma_start(Q[:], q_in[:])

            # ---- per-slot tables ------------------------------------------
            A, TAB, UP = [], [], []
            for r in range(R):
                a = sing.tile([9, P], F32R, tag=f"A{r}")
                nc.sync.dma_start(a[:], tau_in[r])
                A.append(a)
                t = sing.tile([4, P], BF16, tag=f"T{r}")
                nc.sync.dma_start(t[:], tp_in[r])
                TAB.append(t)
                w = sing.tile([128, 48], F32, tag=f"W{r}")
                nc.sync.dma_start(w[:], up_in[r])
                UP.append(w)

            # ---- pre-allocate + touch transpose/master tiles early: pins
            # their arena zones before any release, avoiding alloc-release
            # cycles (a tile whose zone reuses space freed by its own first
            # instruction deadlocks the scheduler). Also zeroes pad columns.
            TG = mats.tile([32, 32], F32, tag="TG")
            TS = mats.tile([32, 32], F32, tag="TS")
            TS1 = mats.tile([32, 32], F32, tag="TS1")
            TSD = mats.tile([32, 32], F32, tag="TSD")
            LT9 = sing.tile([9, 4 * R], F32R, tag="LT9")
            S1ROW = sing.tile([1, 6 * R], F32, tag="S1ROW")
            SDROW = sing.tile([1, 6 * R], F32, tag="SDROW")
            ones1 = sing.tile([1, 128], F32, tag="ones1")
            nc.vector.memset(ones1[:], 1.0)
            zrB = sing.tile([9, 4 * R], F32, tag="zrB")
            nc.vector.memset(zrB[:], 0.0)
            for Tt in (TG, TS, TS1, TSD):
                nc.vector.memset(Tt[:], 0.0)
            nc.scalar.copy(LT9[:], zrB[:])

            # ---- quaternions -> R1, R2, G = -2 R1^T R2, S2 = R2^T R2 ------
            Q2 = sing.tile([32, 8], F32, tag="Q2")
            nc.scalar.activation(Q2[:], Q[:], ACTF.Copy, bias=0.0,
                                 scale=float(np.sqrt(2.0)))
            R1e = mats.tile([32, 32], F32, tag="R1e")
            R2e = mats.tile([32, 32], F32, tag="R2e")
            Ge = mats.tile([32, 32], F32, tag="Ge")
            Se = mats.tile([32, 32], F32, tag="Se")
            S1a = mats.tile([32, 32], F32, tag="S1a")
            SDa = mats.tile([32, 32], F32, tag="SDa")
            RDe = mats.tile([32, 32], F32, tag="RDe")
            for Re, off in ((R1e, 0), (R2e, 4)):
                Pt = Re[:, 16:25]  # scratch: products live in cols 16..24
                s_, u_, v_, w_ = (Q2[:, off + k:off + k + 1] for k in range(4))
                uvw = Q2[:, off + 1:off + 4]
                nc.vector.tensor_mul(Re[:, 16:19], uvw, uvw)      # uu,vv,ww
                nc.vector.tensor_mul(Re[:, 19:20], u_, v_)        # uv
                nc.vector.tensor_mul(Re[:, 20:21], u_, w_)        # uw
                nc.vector.tensor_mul(Re[:, 21:22], v_, w_)        # vw
                nc.vector.tensor_mul(Re[:, 22:23], s_, u_)        # su
                nc.vector.tensor_mul(Re[:, 23:24], s_, v_)        # sv
                nc.vector.tensor_mul(Re[:, 24:25], s_, w_)        # sw
                _emit_quat_entries(nc, Re, Pt)
            # G[i,j] = sum_k R1[k,i] R2[k,j]; entry col 3j+i; then scale -2
            for j in range(3):
                nc.vector.tensor_scalar_mul(
                    Ge[:, 3 * j:3 * j + 3], R1e[:, 0:9:3],
                    R2e[:, 3 * j:3 * j + 1])
                for k in (1, 2):
                    nc.vector.scalar_tensor_tensor(
                        Ge[:, 3 * j:3 * j + 3], R1e[:, k:9:3],
                        R2e[:, 3 * j + k:3 * j + k + 1],
                        Ge[:, 3 * j:3 * j + 3],
                        op0=ALU.mult, op1=ALU.add)
            nc.scalar.activation(Ge[:, 0:9], Ge[:, 0:9], ACTF.Copy,
                                 bias=0.0, scale=-2.0)
            # S2 entries: [S00,S11,S22,2*S01,2*S02,2*S12], S=R2^T R2
            for c, (j, k) in enumerate(((0, 0), (1, 1), (2, 2),
                                        (0, 1), (0, 2), (1, 2))):
                nc.vector.tensor_mul(Se[:, 16:19], R2e[:, 3 * j:3 * j + 3],
                                     R2e[:, 3 * k:3 * k + 3])
                nc.vector.tensor_reduce(Se[:, c:c + 1], Se[:, 16:19],
                                        axis=AX.X, op=ALU.add)
            nc.scalar.activation(Se[:, 3:6], Se[:, 3:6], ACTF.Copy,
                                 bias=0.0, scale=2.0)
            # S1 = R1^T R1 (for n1) and SD = (R1-R2)^T (R1-R2) (for d_same)
            nc.vector.tensor_sub(RDe[:, 0:9], R1e[:, 0:9], R2e[:, 0:9])
            for Sx, Rx in ((S1a, R1e), (SDa, RDe)):
                for c, (j, k) in enumerate(((0, 0), (1, 1), (2, 2),
                                            (0, 1), (0, 2), (1, 2))):
                    nc.vector.tensor_mul(Sx[:, 16:19], Rx[:, 3 * j:3 * j + 3],
                                         Rx[:, 3 * k:3 * k + 3])
                    nc.vector.tensor_reduce(Sx[:, c:c + 1], Sx[:, 16:19],
                                            axis=AX.X, op=ALU.add)
                nc.scalar.activation(Sx[:, 3:6], Sx[:, 3:6], ACTF.Copy,
                                     bias=0.0, scale=2.0)

            # ---- transpose into per-slot lhsT master tiles ----------------
            nc.vector.transpose(TG[:], Ge[:])
            nc.vector.transpose(TS[:], Se[:])
            nc.vector.transpose(TS1[:], S1a[:])
            nc.vector.transpose(TSD[:], SDa[:])
            # LTG[j, 4r+i] = G_r[i,j] (col 4r+3 = 0);
            # LTS[c, 4r+3] = s2_r[c]  (cols 4r..4r+2 = 0);
            # S1ROW/SDROW[0, 6r+c] = quadratic-form weights per slot
            for j in range(3):
                for i in range(3):
                    dst = LT9[j:j + 1, i:4 * R:4]
                    src = TG[3 * j + i:3 * j + i + 1, 0:R].bitcast(F32R)
                    nc.gpsimd.dma_start(dst, src)
            for c in range(6):
                nc.gpsimd.dma_start(LT9[3 + c:4 + c, 3:4 * R:4],
                                    TS[c:c + 1, 0:R].bitcast(F32R))
                nc.gpsimd.dma_start(S1ROW[0:1, c:6 * R:6], TS1[c:c + 1, 0:R])
                nc.gpsimd.dma_start(SDROW[0:1, c:6 * R:6], TSD[c:c + 1, 0:R])

            # ---- constants ------------------------------------------------
            onescol = sing.tile([128, 1], F32, tag="onescol")
            nc.vector.memset(onescol[:], 1.0)
            biasc = sing.tile([128, 1], F32, tag="biasc")
            nc.vector.memset(biasc[:], -MARGIN)
            H = sing.tile([128, 8 * R], F32, tag="H")

            # ---- slot bodies (emitted software-pipelined) -----------------
            def emit_sym_prep(r):
                a = A[r]
                tab = TAB[r]
                # Y4[0:3] = G a ; Y4[3] = s2 . u: one K=9 matmul per half
                # over the merged [a; u] table
                Yb = ybp.tile([4, P], BF16, tag="Yb")
                for n in range(2):
                    nsl = slice(512 * n, 512 * (n + 1))
                    y_ps = ypp.tile([4, 512], F32, tag="y")
                    nc.tensor.matmul(y_ps[:], LT9[0:9, 4 * r:4 * r + 4],
                                     a[0:9, nsl], start=True, stop=True)
                    nc.scalar.copy(Yb[0:4, nsl], y_ps[:])
                # n1[p] = ||R1 a_p||^2 = s1 . u_p (quadratic form): broadcast
                # s1 down partitions via a K=1 matmul, then 6 DVE mult-adds
                up = UP[r]
                s1b = wpp.tile([128, 6], F32, tag="w")
                nc.tensor.matmul(s1b[:], ones1[:],
                                 S1ROW[0:1, 6 * r:6 * r + 6],
                                 start=True, stop=True)
                n1pt = work.tile([128, 8], F32, tag="n1pt")
                nc.vector.tensor_scalar_mul(n1pt[:], up[:, 0:48:6],
                                            s1b[:, 0:1])
                for c in range(1, 6):
                    nc.vector.scalar_tensor_tensor(
                        n1pt[:], up[:, c:48:6], s1b[:, c:c + 1], n1pt[:],
                        op0=ALU.mult, op1=ALU.add)
                return Yb, n1pt

            def emit_sym_tail(r, Yb, n1pt):
                tab = TAB[r]
                mincols = work.tile([128, 8], F32, tag="mincols")
                for t in range(8):
                    pwa = pwp.tile([128, P], F32, tag="pw")
                    fsl = slice(128 * t, 128 * (t + 1))
                    for n in range(2):
                        nsl = slice(512 * n, 512 * (n + 1))
                        nc.tensor.matmul(pwa[:, nsl], tab[0:4, fsl],
                                         Yb[0:4, nsl], start=True, stop=True)
                    nc.vector.tensor_reduce(mincols[:, t:t + 1], pwa[:],
                                            axis=AX.X, op=ALU.min)
                d = work.tile([128, 8], F32, tag="dtile")
                nc.gpsimd.tensor_add(d[:], n1pt[:], mincols[:])
                nc.scalar.activation(H[:, 8 * r:8 * r + 8], d[:], ACTF.Relu,
                                     bias=biasc[:], scale=0.5)

            def emit_ns(r):
                up = UP[r]
                sdb = wpp.tile([128, 6], F32, tag="w")
                nc.tensor.matmul(sdb[:], ones1[:],
                                 SDROW[0:1, 6 * r:6 * r + 6],
                                 start=True, stop=True)
                dts = work.tile([128, 8], F32, tag="nsd")
                nc.vector.tensor_scalar_mul(dts[:], up[:, 0:48:6],
                                            sdb[:, 0:1])
                for c in range(1, 6):
                    nc.vector.scalar_tensor_tensor(
                        dts[:], up[:, c:48:6], sdb[:, c:c + 1], dts[:],
                        op0=ALU.mult, op1=ALU.add)
                nc.scalar.activation(H[:, 8 * r:8 * r + 8], dts[:], ACTF.Relu,
                                     bias=biasc[:], scale=0.5)

            # pipeline: prep slot r+1 is emitted before the heavy tail of
            # slot r; non-symmetric slots are woven between sym tails.
            ns_list = list(range(S, R))
            ns_pos = 0
            stride = max(1, S // max(1, len(ns_list))) if ns_list else 0
            from collections import deque
            pend = deque()
            for i in range(min(3, S)):
                pend.append(emit_sym_prep(i))
            for i in range(S):
                if i + 3 < S:
                    pend.append(emit_sym_prep(i + 3))
                emit_sym_tail(i, *pend.popleft())
                if ns_list and ns_pos < len(ns_list) and stride and \
                        i % stride == stride - 1:
                    emit_ns(ns_list[ns_pos])
                    ns_pos += 1
            for k in range(ns_pos, len(ns_list)):
                emit_ns(ns_list[k])

            # ---- final reduction ------------------------------------------
            colsum = sing.tile([128, 1], F32, tag="colsum")
            nc.vector.tensor_reduce(colsum[:], H[:], axis=AX.X, op=ALU.add)
            ps = wpp.tile([1, 1], F32, tag="w")
            nc.tensor.matmul(ps[:], colsum[:], onescol[:], start=True,
                             stop=True)
            outs = sing.tile([1, 1], F32, tag="outs")
            nc.scalar.copy(outs[:], ps[:])
            nc.gpsimd.dma_start(out_d[:], outs[0, :])
    nc.compile()
    return nc


def _quat_ident():
    return np.array([1.0, 0, 0, 0, 1.0, 0, 0, 0], dtype=np.float32)


def kernel(poses_pred, poses_target, poses_labels, points, symmetry):
    global last_results
    poses_pred = np.asarray(poses_pred, dtype=np.float32)
    poses_target = np.asarray(poses_target, dtype=np.float32)
    poses_labels = np.asarray(poses_labels)
    points = np.asarray(points, dtype=np.float32)
    symmetry = np.asarray(symmetry)

    valid = poses_labels > 0
    is_sym = (symmetry[poses_labels] > 0) & valid
    is_ns = (~(symmetry[poses_labels] > 0)) & valid
    sym_idx = np.nonzero(is_sym)[0]
    ns_idx = np.nonzero(is_ns)[0]

    if len(sym_idx) == 0 and len(ns_idx) == 0:
        return np.float32(0.0)

    S = int(np.ceil(len(sym_idx) / N_CORES))
    NS = int(np.ceil(len(ns_idx) / N_CORES))
    R = S + NS

    key = (S, NS)
    if key not in _program_cache:
        _program_cache[key] = build_program(S, NS)
    nc = _program_cache[key]

    # per-class tables
    ptsT = np.ascontiguousarray(points.transpose(0, 2, 1))  # [C, 3, P]
    x, y, z = ptsT[:, 0], ptsT[:, 1], ptsT[:, 2]
    uq = np.stack([x * x, y * y, z * z, x * y, x * z, y * z], 1)  # [C, 6, P]
    a4 = np.concatenate([ptsT, np.ones((points.shape[0], 1, P), np.float32)],
                        axis=1)  # [C, 4, P]
    tpk = a4
    # partition-layout quadratic features: upk[c][p, 6t+cc] = uq[c][cc, 128t+p]
    upk = np.ascontiguousarray(
        uq.reshape(points.shape[0], 6, 8, 128).transpose(0, 3, 2, 1)
        .reshape(points.shape[0], 128, 48))

    in_maps = []
    for k in range(N_CORES):
        q = np.zeros((32, 8), dtype=np.float32)
        tau = np.empty((R, 9, P), dtype=np.float32)
        tp = np.empty((R, 4, P), dtype=np.float32)  # cast to bf16 below
        upt = np.empty((R, 128, 48), dtype=np.float32)
        my_sym = sym_idx[k::N_CORES]
        my_ns = ns_idx[k::N_CORES]
        for r in range(R):
            if r < len(my_sym):
                src = int(my_sym[r])
            elif S <= r < S + len(my_ns):
                src = int(my_ns[r - S])
            else:
                src = None
            if src is not None:
                lb = int(poses_labels[src])
                q[r, 0:4] = poses_pred[src, lb]
                q[r, 4:8] = poses_target[src, lb]
            else:
                q[r] = _quat_ident()
                lb = 0
            tau[r, 0:3] = ptsT[lb]
            tau[r, 3:9] = uq[lb]
            tp[r] = tpk[lb]
            upt[r] = upk[lb]
        import ml_dtypes
        in_maps.append({"q": q, "tau": tau,
                        "tp": tp.astype(ml_dtypes.bfloat16), "up": upt})

    res = run_bass_kernel_spmd(nc, in_maps, list(range(N_CORES)))
    last_results = res
    total = float(sum(float(res.results[k]["out"][0]) for k in range(N_CORES)))
    return np.float32(total / (B * P))
